# revision 1
# baseline (speedup 1.0000x reference)
"""KNN top-k (K=20, smallest distances) Bass kernel for Trainium2.

Contract: kernel(inputs=np.ndarray[8,4096,4096] fp32) -> np.ndarray[8,4096,20] int32,
identical to jax.lax.top_k(-inputs, 20)[1] including tie semantics (ties broken
toward the lower index).

Sharding: data-parallel over the batch dim — one batch element per NeuronCore,
8 cores. Each core's program: per 128-row tile, negate on the scalar engine,
then 3 rounds of (max8 -> max_index -> match_replace) on the vector engine to
extract the 20 largest of -x per row. max_index's first-occurrence /
sequential-duplicate semantics reproduce jax.lax.top_k tie-breaking exactly.
"""
import numpy as np
from contextlib import ExitStack

import concourse.bacc as bacc
import concourse.tile as tile
from concourse import mybir
from concourse.bass_utils import run_bass_kernel_spmd

B = 8
N = 4096
K = 20
NEG_INF = -1e30

_nc_cache = None


def _build():
    nc = bacc.Bacc("TRN2", target_bir_lowering=False, debug=False, num_devices=B)
    x = nc.dram_tensor("x", [N, N], mybir.dt.float32, kind="ExternalInput")
    y = nc.dram_tensor("y", [N, K], mybir.dt.int32, kind="ExternalOutput")
    ntiles = N // 128
    with tile.TileContext(nc) as tc:
        with ExitStack() as ctx:
            xpool = ctx.enter_context(tc.tile_pool(name="xt", bufs=3))
            wpool = ctx.enter_context(tc.tile_pool(name="wt", bufs=3))
            spool = ctx.enter_context(tc.tile_pool(name="small", bufs=3))
            for t in range(ntiles):
                xt = xpool.tile([128, N], mybir.dt.float32)
                nc.sync.dma_start(out=xt[:], in_=x[t * 128:(t + 1) * 128, :])
                wt = wpool.tile([128, N], mybir.dt.float32)
                # W = -X on the scalar engine; keeps the vector engine free.
                nc.scalar.activation(out=wt[:], in_=xt[:],
                                     func=mybir.ActivationFunctionType.Copy,
                                     scale=-1.0)
                m8 = spool.tile([128, 8], mybir.dt.float32)
                idx = spool.tile([128, 24], mybir.dt.uint32)
                for r in range(3):
                    nc.vector.max(out=m8[:], in_=wt[:])
                    nc.vector.max_index(out=idx[:, r * 8:(r + 1) * 8],
                                        in_max=m8[:], in_values=wt[:])
                    if r < 2:
                        nc.vector.match_replace(out=wt[:], in_to_replace=m8[:],
                                                in_values=wt[:], imm_value=NEG_INF)
                out_t = spool.tile([128, K], mybir.dt.int32)
                nc.vector.tensor_copy(out_t[:], idx[:, :K])
                nc.sync.dma_start(out=y[t * 128:(t + 1) * 128, :], in_=out_t[:])
    nc.compile()
    return nc


def _get_nc():
    global _nc_cache
    if _nc_cache is None:
        _nc_cache = _build()
    return _nc_cache


def kernel(inputs: np.ndarray) -> np.ndarray:
    assert inputs.shape == (B, N, N), inputs.shape
    x = np.ascontiguousarray(np.asarray(inputs, dtype=np.float32))
    nc = _get_nc()
    in_maps = [{"x": x[b]} for b in range(B)]
    res = run_bass_kernel_spmd(nc, in_maps, core_ids=list(range(B)))
    out = np.stack([res.results[b]["y"] for b in range(B)]).astype(np.int32)
    return out
